# revision 37
# baseline (speedup 1.0000x reference)
"""Trainium2 Bass kernel for nn_AttentionBlock (B=8, L=2048, C=512, GroupNorm(8) +
single-head attention + residual), data-parallel over batch across 8 NeuronCores.

Self-contained: hardcodes shapes/sharding. kernel(**inputs) -> np.ndarray [B,L,C].

Two-matmul attention: the four projection weights collapse into two on the host
  W1 = wq @ wk^T / sqrt(C)     (S = h W1 h^T  -- q/k projections fused)
  W2 = wv @ wp                 (attn @ (h W2) -- v/output projections fused)
so the device computes, per core / batch element (channel-major h^T [C, L]):
  x^T fp16 --bn_stats/group-reduce--> a_c, b_c  (rstd via Newton-rsqrt on DVE,
    so the single ACT table set loaded at t=0 serves every activation)
  hb  = a*x + b                  (fp8, matmul operand; also the S lhsT = "K")
  hbf = a*x + b + (bp + bv@wp)   (fp16, residual + folded biases)
  U^T = W1^T h^T  (fp8)  ;  V' = h W2  (fp8, natural [L, C] layout)
  per 512-wide lq tile:
     for each 128-key block kb: S^T = hb-chunk^T @ U^T (PSUM); P = exp(S^T) fp8
     po += V'-chunk^T @ P  (PSUM accum) ; pd += (2^KV 1)^T @ P
     out^T = po * bcast(1/pd) + hbf     (fp16, DMA'd out)
Per-query bias terms cancel in softmax (exact); per-key terms (only if bq != 0)
ride the exp's per-partition bias.
"""

import numpy as np

B, L, C = 8, 2048, 512
GROUPS = 8
EPS = 1e-3
P = 128
CS = C // P            # 4 channel subtiles of 128
LQ = 512               # lq tile width (matmul free dim)
NLT = L // LQ          # 4 lq tiles
NLB = L // P           # 16 key/l blocks
CPG = C // GROUPS      # 64 channels per group
N_CORES = 8

_CACHE = {}


def _build_nc(with_kappa):
    from contextlib import ExitStack

    import concourse.bass as bass
    import concourse.mybir as mybir
    import concourse.tile as tile
    from concourse import bacc
    from concourse.bass import ts

    f32 = mybir.dt.float32
    f32r = mybir.dt.float32r
    fp16 = mybir.dt.float16
    bf16 = mybir.dt.bfloat16
    i32 = mybir.dt.int32
    fp8 = mybir.dt.float8e4
    DR = mybir.MatmulPerfMode.DoubleRow
    AF = mybir.ActivationFunctionType
    ALU = mybir.AluOpType

    nc = bacc.Bacc(trn_type="TRN2")

    # chunk-major [s*4+j, p, 512]: each 128KB DMA reads contiguous DRAM
    xh_d = nc.dram_tensor("xh", [4 * CS, P, 512], fp16, kind="ExternalInput")
    w_d = {
        n: nc.dram_tensor(n, [P, CS, C], fp8, kind="ExternalInput")
        for n in ("w1", "w2")
    }
    # packed per-channel vectors: [gamma, beta, bres] x CS columns
    vp_d = nc.dram_tensor("vp", [P, 3 * CS], f32, kind="ExternalInput")
    g0_d = nc.dram_tensor("g0", [P, 2], f32, kind="ExternalInput")
    sel_d = nc.dram_tensor("sel", [2, P], f32, kind="ExternalInput")
    mg_d = nc.dram_tensor("mg", [2, CS], i32, kind="ExternalInput")
    if with_kappa:
        c3_d = nc.dram_tensor("c3", [P, CS, 1], fp8, kind="ExternalInput")
        ksc_d = nc.dram_tensor("ksc", [P, 2], f32, kind="ExternalInput")
    out_d = nc.dram_tensor("out_t", [C, L], fp16, kind="ExternalOutput")

    out_dv = out_d[:].rearrange("(s p) l -> p s l", p=P)

    # scales (powers of two; host mirrors these exactly)
    KU = 5                 # ut = U * 2^KU
    KV = 2                 # vt = V' * 2^KV ; denom-ones = 2^KV so po/pd cancels

    with tile.TileContext(nc) as tc, ExitStack() as ctx:
        consts = ctx.enter_context(tc.tile_pool(name="consts", bufs=1))
        data = ctx.enter_context(tc.tile_pool(name="data", bufs=1))
        small = ctx.enter_context(tc.tile_pool(name="small", bufs=1))
        ptp = ctx.enter_context(tc.tile_pool(name="ptp", bufs=4))
        t1p = ctx.enter_context(tc.tile_pool(name="t1p", bufs=2))
        outp = ctx.enter_context(tc.tile_pool(name="outp", bufs=4))
        finp = ctx.enter_context(tc.tile_pool(name="finp", bufs=2))
        psA = ctx.enter_context(tc.tile_pool(name="psA", bufs=4, space="PSUM"))
        psS = ctx.enter_context(tc.tile_pool(name="psS", bufs=3, space="PSUM"))
        psD = ctx.enter_context(tc.tile_pool(name="psD", bufs=1, space="PSUM"))

        # ---- SBUF residents ----
        xh = data.tile([P, CS, L], fp16)      # x^T fp16
        hb = data.tile([P, CS, L], fp8)       # h^T fp8 (matmul operand + S lhsT)
        hbf = data.tile([P, CS, L], fp16)     # h^T + bres (residual, fp16)
        ut = data.tile([P, CS, L], fp8)       # U^T * 2^KU
        vt = data.tile([P, NLB, C], fp8)      # V' natural, [l%P, l//P, c] * 2^KV
        wsb = {n: consts.tile([P, CS, C], fp8, name=f"w_{n}") for n in w_d}
        vp = consts.tile([P, 3 * CS], f32)
        GAM, BET, BRES = (vp[:, i * CS:(i + 1) * CS] for i in range(3))
        g0 = consts.tile([P, 2], f32)
        sel = consts.tile([2, P], f32)
        ones_col = consts.tile([P, 2, 16], fp8)   # [:, :, 0:1] = 2^KV (DR pair)
        ones_row = consts.tile([1, P], bf16)
        eps2 = consts.tile([2, 1], f32)
        magic = consts.tile([2, CS], i32)
        if with_kappa:
            c3v = consts.tile([P, CS, 1], fp8)
            kscv = consts.tile([P, 2], f32)
            ksb = small.tile([P, NLB], f32)

        # ---- loads + constants ----
        # 512-wide chunks round-robined over the three DMA-capable queues so
        # bn_stats starts on the first chunk ~1us after the preamble.
        nc.gpsimd.dma_start(out=g0[:], in_=g0_d[:])
        nc.gpsimd.dma_start(out=sel[:], in_=sel_d[:])
        nc.gpsimd.dma_start(out=vp[:], in_=vp_d[:])
        nc.gpsimd.dma_start(out=magic[:], in_=mg_d[:])
        if with_kappa:
            nc.gpsimd.dma_start(out=c3v[:], in_=c3_d[:])
            nc.gpsimd.dma_start(out=kscv[:], in_=ksc_d[:])

        def xchunk(q, s, j):
            q.dma_start(out=xh[:, s, ts(j, 512)], in_=xh_d[4 * s + j])

        # per-queue order: first chunk of each bn_stats subtile leads (it
        # gates the DVE pipeline), s3 chunks next (ACT accum path)
        for j in range(4):
            xchunk(nc.sync, 0, j)
            xchunk(nc.scalar, 1, j)
            xchunk(nc.gpsimd, 2, j)
            if j == 0:
                xchunk(nc.sync, 3, 0)
                xchunk(nc.scalar, 3, 1)
                xchunk(nc.gpsimd, 3, 2)
            if j == 1:
                xchunk(nc.sync, 3, 3)
        nc.scalar.dma_start(out=wsb["w1"][:], in_=w_d["w1"][:])
        nc.scalar.dma_start(out=wsb["w2"][:], in_=w_d["w2"][:])
        nc.vector.memset(ones_col[:], float(2.0 ** KV))
        nc.vector.memset(ones_row[:], 1.0)
        nc.vector.memset(eps2[:], EPS)
        # warm the single ACT table set (exp_and_others: exp/identity/square)
        dm = small.tile([2, 1], f32, name="dm")
        nc.scalar.activation(out=dm[:], in_=eps2[:], func=AF.Exp)

        # ---- GroupNorm stats ----
        # per-channel (partition) sum / sumsq over L: subtiles 0-2 via DVE
        # bn_stats (chunk order matches DMA arrival); subtile 3 via ACT
        # activation accum (Identity / Square) on 1024-wide halves.
        st = small.tile([P, CS, 2], f32)      # (mean_c, E[x^2]_c) per subtile
        st6 = small.tile([P, 3, 4, 6], f32)
        for j in range(4):
            for s in range(3):
                if s == 2 and j >= 2:
                    continue  # s2's tail chunks go to the ACT accum path
                nc.vector.bn_stats(out=st6[:, s, j, :], in_=xh[:, s, ts(j, 512)])
        gscr = small.tile([P, L], fp16)
        acc = small.tile([P, 2, 2], f32)      # [p, (s3, s2-half), (sum, sumsq)]
        nc.scalar.activation(out=gscr[:], in_=xh[:, 3, :], func=AF.Identity,
                             accum_out=acc[:, 0, 0:1])
        nc.scalar.activation(out=gscr[:], in_=xh[:, 3, :], func=AF.Square,
                             accum_out=acc[:, 0, 1:2])
        nc.scalar.activation(out=gscr[:, 0:1024], in_=xh[:, 2, 1024:2048], func=AF.Identity,
                             accum_out=acc[:, 1, 0:1])
        nc.scalar.activation(out=gscr[:, 0:1024], in_=xh[:, 2, 1024:2048], func=AF.Square,
                             accum_out=acc[:, 1, 1:2])
        for s in range(2):
            mv = small.tile([P, 2], f32, tag="mv", bufs=2)
            nc.vector.bn_aggr(out=mv[:], in_=st6[:, s, :, :])
            nc.vector.tensor_copy(out=st[:, s, 0:1], in_=mv[:, 0:1])
            nc.vector.tensor_tensor(out=st[:, s, 1:2], in0=mv[:, 0:1], in1=mv[:, 0:1], op=ALU.mult)
            nc.vector.tensor_tensor(out=st[:, s, 1:2], in0=st[:, s, 1:2], in1=mv[:, 1:2], op=ALU.add)
        # subtile 2 = bn_aggr over its first half + ACT accum of the second
        mv2 = small.tile([P, 2], f32)
        nc.vector.bn_aggr(out=mv2[:], in_=st6[:, 2, 0:2, :])
        nc.vector.tensor_tensor(out=st[:, 2, 0:1], in0=mv2[:, 0:1], in1=mv2[:, 0:1], op=ALU.mult)
        nc.vector.tensor_tensor(out=st[:, 2, 0:1], in0=st[:, 2, 0:1], in1=mv2[:, 1:2], op=ALU.add)
        # E[x^2]_s2 = (1024*(m^2+v) + sumsq_half2) / 2048 ; mean likewise
        nc.vector.tensor_scalar(out=st[:, 2, 0:1], in0=st[:, 2, 0:1], scalar1=1024.0,
                                scalar2=acc[:, 1, 1:2], op0=ALU.mult, op1=ALU.add)
        nc.vector.tensor_scalar(out=st[:, 2, 1:2], in0=st[:, 2, 0:1], scalar1=1.0 / L,
                                scalar2=None, op0=ALU.mult)
        nc.vector.tensor_scalar(out=st[:, 2, 0:1], in0=mv2[:, 0:1], scalar1=1024.0,
                                scalar2=acc[:, 1, 0:1], op0=ALU.mult, op1=ALU.add)
        nc.vector.tensor_scalar(out=st[:, 2, 0:1], in0=st[:, 2, 0:1], scalar1=1.0 / L,
                                scalar2=None, op0=ALU.mult)
        nc.vector.tensor_scalar(out=st[:, 3, :], in0=acc[:, 0, :], scalar1=1.0 / L,
                                scalar2=None, op0=ALU.mult)

        psg = psD.tile([2, 2 * CS], f32, tag="d")   # [group-half, (s, stat)]
        nc.tensor.matmul(psg[:], lhsT=g0[:], rhs=st[:].rearrange("p a b -> p (a b)"),
                         start=True, stop=True)
        pst = small.tile([2, 2 * CS], f32)
        nc.vector.tensor_copy(out=pst[:], in_=psg[:])
        pstv = pst[:].rearrange("p (s k) -> p s k", k=2)
        msq = small.tile([2, CS], f32)
        nc.vector.tensor_tensor(out=msq[:], in0=pstv[:, :, 0], in1=pstv[:, :, 0], op=ALU.mult)
        grp = small.tile([2, 2 * CS], f32)     # [:, :CS]=rstd_g, [:, CS:]=mean_g
        vv = small.tile([2, CS], f32)          # var + eps
        nc.vector.tensor_tensor(out=vv[:], in0=pstv[:, :, 1], in1=msq[:], op=ALU.subtract)
        nc.vector.tensor_scalar(out=vv[:], in0=vv[:], scalar1=1.0, scalar2=EPS,
                                op0=ALU.mult, op1=ALU.add)
        # rstd = rsqrt(var+eps): Quake bit-trick seed + 2 Newton iterations,
        # entirely on DVE -- no Sqrt/Ln table switches on the ACT engine.
        nc.vector.tensor_copy(out=grp[:, CS:], in_=pstv[:, :, 0])
        y = grp[:, 0:CS]
        yi = y.bitcast(i32)
        nc.vector.tensor_scalar(out=yi, in0=vv[:].bitcast(i32), scalar1=1, scalar2=None,
                                op0=ALU.logical_shift_right)
        nc.vector.tensor_tensor(out=yi, in0=magic[:], in1=yi, op=ALU.subtract)
        tn = small.tile([2, CS], f32)
        for _ in range(2):
            nc.vector.tensor_tensor(out=tn[:], in0=vv[:], in1=y, op=ALU.mult)
            nc.vector.tensor_tensor(out=tn[:], in0=tn[:], in1=y, op=ALU.mult)
            nc.vector.tensor_scalar(out=tn[:], in0=tn[:], scalar1=-0.5, scalar2=1.5,
                                    op0=ALU.mult, op1=ALU.add)
            nc.vector.tensor_tensor(out=y, in0=y, in1=tn[:], op=ALU.mult)

        psbc = psD.tile([P, 2 * CS], f32, tag="d")  # broadcast groups -> channels
        nc.tensor.matmul(psbc[:], lhsT=sel[:], rhs=grp[:], start=True, stop=True)
        ab = small.tile([P, 2 * CS], f32)      # [:, :CS]=a_c, [:, CS:]=b_c
        nc.vector.tensor_tensor(out=ab[:, 0:CS], in0=GAM, in1=psbc[:, 0:CS], op=ALU.mult)
        nc.vector.tensor_tensor(out=ab[:, CS:], in0=psbc[:, CS:], in1=ab[:, 0:CS], op=ALU.mult)
        nc.vector.tensor_tensor(out=ab[:, CS:], in0=BET, in1=ab[:, CS:], op=ALU.subtract)
        # residual-pass intercept: b + bres (bres = bp + bv @ wp, host-folded)
        ab2 = small.tile([P, CS], f32)
        nc.vector.tensor_tensor(out=ab2[:], in0=ab[:, CS:], in1=BRES, op=ALU.add)

        # ---- normalize ----
        # hb (fp8 matmul operand) per (subtile, lt-slice) so the first U/V'
        # matmuls start after 4 small ops; split DVE/ACT. hbf (fp16 residual)
        # only feeds finales: lt0 early on DVE, lt1-3 on GpSimd.
        def hb_slice(s, lt):
            if (s + lt) % 2 == 0:
                nc.vector.tensor_scalar(out=hb[:, s, ts(lt, LQ)], in0=xh[:, s, ts(lt, LQ)],
                                        scalar1=ab[:, s:s + 1], scalar2=ab[:, CS + s:CS + s + 1],
                                        op0=ALU.mult, op1=ALU.add)
            else:
                nc.scalar.activation(out=hb[:, s, ts(lt, LQ)], in_=xh[:, s, ts(lt, LQ)],
                                     func=AF.Identity,
                                     bias=ab[:, CS + s:CS + s + 1], scale=ab[:, s:s + 1])

        # only lt0's hb slices gate the first U/V'/S matmuls; lt1/lt2 slices
        # are emitted inside lt0's kp loop (DVE+ACT slack), lt3 + all of hbf
        # on GpSimd (first needed at kp6 / the first finale respectively)
        for s in range(CS):
            hb_slice(s, 0)
        for lt in range(NLT):
            for s in range(CS):
                nc.gpsimd.tensor_scalar(out=hbf[:, s, ts(lt, LQ)], in0=xh[:, s, ts(lt, LQ)],
                                        scalar1=ab[:, s:s + 1], scalar2=ab2[:, s:s + 1],
                                        op0=ALU.mult, op1=ALU.add)

        # ---- projections ----
        epi_ix = [0]

        def epi(dst, src, scl, eng):
            # PSUM -> SBUF fp8 cast with scale. 'alt' alternates DVE / ACT
            # (pre-attention, both engines have slack); 'dve' keeps the ACT
            # queue free for the exp stream during attention.
            if eng == "alt":
                eng = "dve" if epi_ix[0] % 2 == 0 else "act"
                epi_ix[0] += 1
            if eng == "dve":
                nc.vector.tensor_scalar(out=dst, in0=src, scalar1=scl, scalar2=None,
                                        op0=ALU.mult)
            else:
                nc.scalar.activation(out=dst, in_=src, func=AF.Identity, scale=scl)

        def u_proj(lt, eng="alt"):
            # ut[:, co_s, lq] = sum_ci W1[ci, co]^T h^T ; scale 2^KU / W1SC
            for co_s in range(CS):
                ps = psS.tile([P, LQ], f32, tag="s", name="ps_u")
                for cp in range(2):
                    nc.tensor.matmul(ps[:], lhsT=wsb["w1"][:, 2 * cp:2 * cp + 2, ts(co_s, P)],
                                     rhs=hb[:, 2 * cp:2 * cp + 2, ts(lt, LQ)],
                                     start=(cp == 0), stop=(cp == 1), perf_mode=DR)
                epi(ut[:, co_s, ts(lt, LQ)], ps[:], float(2.0 ** KU) / W1SC, eng)

        def v_proj(lb, eng="alt"):
            # vt[l%P, lb, :] = (h W2)*2^KV rows for key block lb
            ps = psS.tile([P, C], f32, tag="s", name="ps_v")
            for cp in range(2):
                nc.tensor.matmul(ps[:], lhsT=hb[:, 2 * cp:2 * cp + 2, ts(lb, P)],
                                 rhs=wsb["w2"][:, 2 * cp:2 * cp + 2, :],
                                 start=(cp == 0), stop=(cp == 1), perf_mode=DR)
            epi(vt[:, lb, :], ps[:], float(2.0 ** KV) / W2SC, eng)

        if with_kappa:
            # kappa[m] = h_m . c3 + const  (per-key exp bias; only if bq != 0)
            psk = psS.tile([P, NLB], f32, tag="s", name="ps_k")
            for kb in range(NLB):
                for cp in range(2):
                    nc.tensor.matmul(psk[:, kb:kb + 1],
                                     lhsT=hb[:, 2 * cp:2 * cp + 2, ts(kb, P)],
                                     rhs=c3v[:, 2 * cp:2 * cp + 2, :],
                                     start=(cp == 0), stop=(cp == 1), perf_mode=DR)
            nc.vector.tensor_scalar(out=ksb[:], in0=psk[:], scalar1=kscv[:, 0:1],
                                    scalar2=kscv[:, 1:2], op0=ALU.mult, op1=ALU.add)

        u_proj(0)
        for lb in range(4):
            v_proj(lb)

        # ---- attention, per lq tile ----
        EXPS = float(2.0 ** -KU)

        def finale(lt, po, pd):
            # pd = 2^KV * denom ; rb = bcast(1/pd) ; out = po*rb + hbf
            # pdc on ACT: the DVE queue lags at finale time, ACT has slack
            pdc = small.tile([1, LQ], bf16, tag="pdc", bufs=2)
            with nc.allow_low_precision(reason="denom rounded to bf16 as matmul operand"):
                nc.scalar.activation(out=pdc[:], in_=pd[:], func=AF.Identity)
            pb = psS.tile([P, LQ], f32, tag="s", name="ps_b")
            nc.tensor.matmul(pb[:], lhsT=ones_row[:], rhs=pdc[:], start=True, stop=True)
            rb = finp.tile([P, LQ], f32, tag="rb")
            nc.vector.reciprocal_approx_fast(out=rb[:], in_=pb[:])
            for c_ in range(CS):
                t1 = t1p.tile([P, LQ], fp16, tag="t1")
                with nc.allow_low_precision(reason="attn term to fp16"):
                    nc.vector.tensor_tensor(out=t1[:], in0=po[c_][:], in1=rb[:], op=ALU.mult)
                ot = outp.tile([P, LQ], fp16, tag="ot")
                nc.vector.tensor_tensor(out=ot[:], in0=t1[:], in1=hbf[:, c_, ts(lt, LQ)],
                                        op=ALU.add)
                # sync queue only: a dependent dma_start on the ACT queue
                # would stall the exp instruction stream at the wait
                nc.sync.dma_start(out=out_dv[:, c_, ts(lt, LQ)], in_=ot[:])

        pending = None  # (lt, po, pd) awaiting finale emission
        for lt in range(NLT):
            po = [psA.tile([P, LQ], f32, tag="po", name=f"po{i}") for i in range(CS)]
            pd = psD.tile([1, LQ], f32, tag="d", name="pd")

            def pv_group(kp, pt2):
                for c_ in range(CS):
                    nc.tensor.matmul(po[c_][:], lhsT=vt[:, 2 * kp:2 * kp + 2, ts(c_, P)],
                                     rhs=pt2[:], start=(kp == 0), stop=(kp == NLB // 2 - 1),
                                     perf_mode=DR)
                nc.tensor.matmul(pd[:], lhsT=ones_col[:, :, 0:1], rhs=pt2[:],
                                 start=(kp == 0), stop=(kp == NLB // 2 - 1), perf_mode=DR)

            pvq = []   # software pipeline: PV trails S/exp by two kps
            for kp in range(NLB // 2):
                pt2 = ptp.tile([P, 2, LQ], fp8, tag="pt")
                for i in range(2):
                    kb = 2 * kp + i
                    ps = psS.tile([P, LQ], f32, tag="s", name="ps_s")
                    for cp in range(2):
                        nc.tensor.matmul(ps[:], lhsT=hb[:, 2 * cp:2 * cp + 2, ts(kb, P)],
                                         rhs=ut[:, 2 * cp:2 * cp + 2, ts(lt, LQ)],
                                         start=(cp == 0), stop=(cp == 1), perf_mode=DR)
                    if with_kappa:
                        nc.scalar.activation(out=pt2[:, i, :], in_=ps[:], func=AF.Exp,
                                             scale=EXPS, bias=ksb[:, kb:kb + 1])
                    else:
                        nc.scalar.activation(out=pt2[:, i, :], in_=ps[:], func=AF.Exp,
                                             scale=EXPS)
                if kp == 0 and pending is not None:
                    # previous tile's finale: emitted after this tile's first
                    # S pair so the PE has queued work during the DVE chain
                    finale(*pending)
                    pending = None
                if lt == 0 and kp in (1, 2, 3):
                    # trailing hb slices (lt1 at kp1, lt2 at kp2, lt3 at kp3),
                    # needed by S/V'/u_proj from kp2/kp4/kp6 onward -- must be
                    # emitted BEFORE u_proj(lt+1) below, which reads them
                    for s in range(CS):
                        hb_slice(s, kp)
                if kp == 1 and lt < NLT - 1:
                    # next tile's U slice streamed into this tile's PE slack
                    u_proj(lt + 1, eng="dve")
                pvq.append((kp, pt2))
                if len(pvq) > 2:
                    pv_group(*pvq.pop(0))
                if lt == 0 and kp >= 2:
                    # V' blocks 4..15 interleaved two per kp; PV(kp) only
                    # needs blocks 2kp..2kp+1, produced one kp ahead
                    v_proj(2 * kp, eng="dve")
                    v_proj(2 * kp + 1, eng="dve")
            for g in pvq:
                pv_group(*g)
            pending = (lt, po, pd)
        finale(*pending)

    nc.compile()
    return nc


def get_nc(with_kappa):
    key = ("nc", with_kappa, W1SC, W2SC)
    if key not in _CACHE:
        _CACHE[key] = _build_nc(with_kappa)
    return _CACHE[key]


def _g0_const():
    g = np.zeros((P, 2), np.float32)
    g[0:CPG, 0] = 1.0 / CPG
    g[CPG:P, 1] = 1.0 / CPG
    return g


def _sel_const():
    s = np.zeros((2, P), np.float32)
    s[0, 0:CPG] = 1.0
    s[1, CPG:P] = 1.0
    return s


def _pow2_scale(w, target=240.0):
    """Largest power-of-2 s with max|w|*s <= target (fp8e4m3 headroom)."""
    m = float(np.abs(w).max())
    if m == 0.0:
        return 1.0
    return float(2.0 ** np.floor(np.log2(target / m)))


# module-level so _build_nc sees the host-chosen weight scales
W1SC = 1.0
W2SC = 1.0


def prep_inputs(x, gamma, beta, wq, bq, wk, bk, wv, bv, wp, bp):
    """Host-side prep: fold wq@wk^T and wv@wp, transpose/cast, per-core maps."""
    global W1SC, W2SC
    import ml_dtypes

    f = np.float32
    f8 = ml_dtypes.float8_e4m3fn
    x = np.asarray(x, f)
    lam = f(C) ** f(-0.5)

    w1 = (np.asarray(wq, f) @ np.asarray(wk, f).T) * lam      # S = h w1 h^T
    w2 = np.asarray(wv, f) @ np.asarray(wp, f)                # o' = h w2
    bres = np.asarray(bp, f) + np.asarray(bv, f) @ np.asarray(wp, f)
    W1SC = _pow2_scale(w1)
    W2SC = _pow2_scale(w2)
    with_kappa = bool(np.any(np.asarray(bq, f)))

    def wprep(w, s):
        w = np.asarray(w, f) * s
        return np.ascontiguousarray(w.reshape(CS, P, C).transpose(1, 0, 2)).astype(f8)

    def vprep(v):
        v = np.asarray(v, f)
        return np.ascontiguousarray(v.reshape(CS, P).T)

    shared = {
        "w1": wprep(w1, W1SC), "w2": wprep(w2, W2SC),
        "vp": np.ascontiguousarray(np.concatenate(
            [vprep(gamma), vprep(beta), vprep(bres)], axis=1)),
        "g0": _g0_const(), "sel": _sel_const(),
        "mg": np.full((2, CS), 0x5F3759DF, np.int32),
    }
    if with_kappa:
        # kappa[m] = lam * (h_m wk) . bq + lam * bq.bk  (per-key exp bias)
        c3 = lam * (np.asarray(wk, f) @ np.asarray(bq, f))    # [C]
        c3s = _pow2_scale(c3)
        shared["c3"] = np.ascontiguousarray(
            (c3 * c3s).reshape(CS, P).T.reshape(P, CS, 1)).astype(f8)
        kconst = lam * float(np.asarray(bq, f) @ np.asarray(bk, f))
        ksc = np.empty((P, 2), f)
        ksc[:, 0] = 1.0 / c3s
        ksc[:, 1] = kconst
        shared["ksc"] = ksc
    in_maps = []
    for b in range(N_CORES):
        m = dict(shared)
        # chunk-major [s*4+j, p, 512] so each 128KB chunk DMA is contiguous
        xt = x[b].T.astype(np.float16)                             # [C, L]
        m["xh"] = np.ascontiguousarray(
            xt.reshape(CS, P, 4, 512).transpose(0, 2, 1, 3).reshape(4 * CS, P, 512))
        in_maps.append(m)
    return in_maps, with_kappa


def run(inputs, trace=False, **kw):
    from concourse.bass_utils import run_bass_kernel_spmd

    in_maps, with_kappa = prep_inputs(**inputs)
    nc = get_nc(with_kappa)
    return run_bass_kernel_spmd(nc, in_maps, core_ids=list(range(N_CORES)),
                                trace=trace, **kw)


def kernel(**inputs) -> np.ndarray:
    res = run(inputs)
    out = np.empty((B, L, C), np.float32)
    for b in range(N_CORES):
        out[b] = np.asarray(res.results[b]["out_t"], np.float32).T
    return out
